# revision 1
# baseline (speedup 1.0000x reference)
"""Trainium2 Bass kernel for nn_CacheAttention (retrieval KNN attention).

Reference computation (per token, fully independent across tokens):
    q = (hidden @ Wq.T) * D**-0.5          # [t, H*D] viewed [t, KV, G, D]
    k = retrieved @ Wk.T                   # [t, N, KV*D] viewed [t, KV, N, D]
    v = retrieved @ Wv.T                   # viewed [t, KV, N, D]
    s = einsum('kgd,knd->kgn', q_t, k_t);  a = softmax(s, -1)
    out_t = einsum('kgn,knd->kgd', a, v_t).reshape(H*D) @ Wo.T

Strategy: data-parallel over the 4096 (b, s) tokens across 8 NeuronCores
(512 tokens each).  The host pre-transposes + bf16-casts all operands so
every matmul contracts over the SBUF partition dim with cheap, contiguous
DMA loads.  Attention uses a block-diagonal PE trick: scores for 32 tokens
x 4 groups in one PSUM tile [(g,t)=128, (t',n)=512], additive -30 mask +
exp (denominator via ACT accum), then a block-diag matmul of V^T against
A^T for the weighted sum.  A^T and V^T come from XBAR DMA-transposes
(InstDmaTransposeAnt) instead of PE transpose+copy, freeing the PE and
DVE.  DMA emission order front-loads the first wq slab + hT chunks and
prefetches rtile-0 / the first Wo sub-slabs to keep the PE dense across
stage boundaries.
"""

import os
import sys

import numpy as np
import ml_dtypes

for _p in ("/opt/trn_rl_repo", "/root/.axon_site/_ro/trn_rl_repo"):
    if os.path.isdir(_p) and _p not in sys.path:
        sys.path.insert(0, _p)

import concourse.bass as bass  # noqa: E402
import concourse.mybir as mybir  # noqa: E402
import concourse.tile as tile  # noqa: E402
from concourse import bacc  # noqa: E402
from concourse.bass_utils import run_bass_kernel_spmd  # noqa: E402

# Problem shapes (hardcoded per contest contract).
B, S, HID = 2, 2048, 4096
H, KV, D = 32, 8, 128
G = H // KV  # 4
N = 16
RH = HID // 4  # 1024
NCORES = 8
TOK = B * S  # 4096 tokens total
T = TOK // NCORES  # 512 tokens per core
TBLK = 32  # tokens per pipeline block
NBLK = T // TBLK  # 16
TN = T * N  # 8192 (token, neighbor) rows per core
TNBLK = TBLK * N  # 512
KC = HID // 128  # 32 contraction chunks for Q/O projections
RC = RH // 128  # 8 contraction chunks for K/V projections
SCALE = float(D) ** -0.5
MASK_NEG = -30.0

BF16 = mybir.dt.bfloat16
F32 = mybir.dt.float32
EXP = mybir.ActivationFunctionType.Exp

_NC = None


def _build_program(reps=1):
    nc = bacc.Bacc(None, target_bir_lowering=False, debug=False)

    hT = nc.dram_tensor("hT", [HID, T], BF16, kind="ExternalInput")
    rT = nc.dram_tensor("rT", [RH, TN], BF16, kind="ExternalInput")
    wqT = nc.dram_tensor("wqT", [HID, H * D], BF16, kind="ExternalInput")
    wkT = nc.dram_tensor("wkT", [RH, KV * D], BF16, kind="ExternalInput")
    wvT = nc.dram_tensor("wvT", [RH, KV * D], BF16, kind="ExternalInput")
    woT = nc.dram_tensor("woT", [H * D, HID], BF16, kind="ExternalInput")
    mneg = nc.dram_tensor("mneg", [128, 512], F32, kind="ExternalInput")
    out = nc.dram_tensor("out", [T, HID], F32, kind="ExternalOutput")

    hT_r = hT[:].rearrange("(c p) t -> p c t", p=128)
    rT_r = rT[:].rearrange("(c p) t -> p c t", p=128)
    wq_r = wqT[:].rearrange("(c p) m -> p c m", p=128)
    wk_r = wkT[:].rearrange("(c p) m -> p c m", p=128)
    wv_r = wvT[:].rearrange("(c p) m -> p c m", p=128)
    wo_r = woT[:].rearrange("(c p) h -> p c h", p=128)
    out_r = out[:].rearrange("(mt p) h -> p mt h", p=128)

    with tile.TileContext(nc) as tc:
      for _rep in range(reps):  # >1 only for timing calibration builds
        with (
            tc.tile_pool(name="resident", bufs=1) as resp,
            tc.tile_pool(name="rt", bufs=2) as rtp,
        ):
            mneg_sb = resp.tile([128, 512], F32)
            wk_sb = resp.tile([128, RC, KV * D], BF16)
            wv_sb = resp.tile([128, RC, KV * D], BF16)
            # Filled by stage 1 / stage 2, consumed downstream.
            # Q^T layout [d, kv, (grp, g, t32)]: the scores stationary for
            # (kv, grp) is then a contiguous [128, 128] slice covering all
            # 4 groups g x 32 tokens (walrus rejects 3D weight APs).
            qT_sb = resp.tile([128, KV, G * T], BF16)
            qT_w = qT_sb[:].rearrange("p h (a g t) -> p h a g t", g=G, t=32)
            aoT_sb = resp.tile([128, H * D // 128, T], BF16)

            # ---- Stage 1: Q^T[(kv,g,d), t] = WqT.T-chunks x hT ----
            with (
                tc.tile_pool(name="hpool", bufs=1) as hp,
                tc.tile_pool(name="wq", bufs=12) as wqp,
                tc.tile_pool(name="ps1", bufs=2, space="PSUM") as ps1,
            ):
                hT_sb = hp.tile([128, KC, T], BF16)
                rt0 = rtp.tile([128, RC, TNBLK], BF16, tag="rt")
                for ms in range(8):  # 512-col slabs of Wq^T
                    slabs = []
                    for kg in range(KC // 4):
                        # 4-chunk-batched transfers: ms=0 needs 64 tiles and
                        # is DMA-bound with per-transfer overheads; batching
                        # keeps stage-1 start compute-bound.
                        sl = wqp.tile([128, 4, 512], BF16, tag="wqslab")
                        nc.sync.dma_start(
                            sl[:], wq_r[:, 4 * kg : 4 * kg + 4, ms * 512 : (ms + 1) * 512]
                        )
                        if ms == 0:
                            nc.sync.dma_start(
                                hT_sb[:, 4 * kg : 4 * kg + 4, :],
                                hT_r[:, 4 * kg : 4 * kg + 4, :],
                            )
                        slabs.append(sl)
                    if ms == 1:
                        # Prefetch block 0's retrieved states + stage-2
                        # weights while Q-projection compute runs (after the
                        # DMA-bound ms=0 group so they don't delay its slabs).
                        nc.sync.dma_start(rt0[:], rT_r[:, :, 0:TNBLK])
                        nc.sync.dma_start(mneg_sb[:], mneg[:])
                    elif ms == 2:
                        nc.sync.dma_start(wk_sb[:], wk_r)
                        nc.sync.dma_start(wv_sb[:], wv_r)
                    for mi in range(4):
                        m = ms * 4 + mi
                        qps = ps1.tile([128, 512], F32, tag="qps")
                        for k in range(KC):
                            nc.tensor.matmul(
                                qps[:],
                                slabs[k // 4][:, k % 4, mi * 128 : (mi + 1) * 128],
                                hT_sb[:, k, :],
                                start=(k == 0),
                                stop=(k == KC - 1),
                            )
                        # Fold the D**-0.5 query scaling into the PSUM evict.
                        nc.scalar.mul(
                            qT_w[:, m // G, :, m % G, :],
                            qps[:].rearrange("p (a t) -> p a t", t=32),
                            SCALE,
                        )

            # ---- Stage 2: per 32-token block: K/V projections + attention ----
            # Software-pipelined: block b's softmax chain (scores -> mask ->
            # exp -> normalize -> A^T DMA-transpose) is kicked off around its
            # K/V projections, and the AV matmuls of block b-1 run inside
            # block b's PE stream, so the ~2us non-PE chain never stalls PE.
            with (
                tc.tile_pool(name="kt", bufs=2) as ktp,
                tc.tile_pool(name="vt", bufs=2) as vtp,
                tc.tile_pool(name="attn", bufs=4) as atp,
                tc.tile_pool(name="atT", bufs=16) as attp,
                tc.tile_pool(name="ps2", bufs=2, space="PSUM") as ps2,
                tc.tile_pool(name="pss", bufs=4, space="PSUM") as pss,
                tc.tile_pool(name="psav", bufs=2, space="PSUM") as psav,
            ):

                def softmax_quad(blk, ktile, kvs, at_list):
                    # Scores psum [(g,t)=128, (t',n)=512]; only the block-
                    # diagonal (t'==t) 16-col slices are valid.  The
                    # stationary packs all 4 GQA groups x 32 tokens as one
                    # contiguous [128, 128] slice of the (grp, g, t) Q layout.
                    for kv in kvs:
                        sps = pss.tile([128, 512], F32, tag="sps")
                        nc.tensor.matmul(
                            sps[:],
                            qT_sb[:, kv, blk * 128 : (blk + 1) * 128],
                            ktile[:, kv, :],
                            start=True,
                            stop=True,
                        )
                        sm = atp.tile([128, 512], F32, tag="sm")
                        nc.vector.tensor_add(sm[:], sps[:], mneg_sb[:])
                        e = atp.tile([128, 512], BF16, tag="e")
                        den = atp.tile([128, 1], F32, tag="den")
                        nc.scalar.activation(e[:], sm[:], EXP, accum_out=den[:])
                        rec = atp.tile([128, 1], F32, tag="rec")
                        nc.vector.reciprocal(rec[:], den[:])
                        a = atp.tile([128, 512], BF16, tag="a")
                        nc.vector.tensor_scalar_mul(a[:], e[:], rec[:])
                        # A^T chunks [(t',n)=128, 4, (g,t)=128] via XBAR
                        # DMA-transpose, consumed one block later.
                        at = attp.tile([128, 4, 128], BF16, tag="at")
                        nc.sync.dma_start(at[:, :, :], a[:], transpose=True)
                        at_list.append(at)

                def attn_weighted_v(blk, vflat, at_list):
                    # attnout^T[d, (g,t)] = sum_c V_chunk.T @ A^T_chunk.
                    for kv in range(KV):
                        avps = psav.tile([128, 128], F32, tag="av")
                        for c in range(4):
                            nc.tensor.matmul(
                                avps[:],
                                vflat[:, kv, c, :],
                                at_list[kv][:, c, :],
                                start=(c == 0),
                                stop=(c == 3),
                            )
                        nc.vector.tensor_copy(
                            aoT_sb[
                                :,
                                kv * G : (kv + 1) * G,
                                blk * 32 : blk * 32 + 32,
                            ],
                            avps[:].rearrange("p (g t) -> p g t", g=G),
                        )

                prev = None
                for blk in range(NBLK):
                    if blk == 0:
                        rtile = rt0
                    else:
                        rtile = rtp.tile([128, RC, TNBLK], BF16, tag="rt")
                        nc.sync.dma_start(
                            rtile[:], rT_r[:, :, blk * TNBLK : (blk + 1) * TNBLK]
                        )

                    # The reference's torch flat-view [t,n,KV*D] -> [t,KV,n,D]
                    # means head kv attends slot nn drawn from neighbor
                    # n_src = 2*kv + nn//8 with kv-slice kvc = nn%8.  Softmax
                    # is permutation-invariant per head, so we only need a
                    # consistent slot order for K and V: slot = (n_src%2)*8
                    # + kvc, gathered on the free dim during PSUM eviction.
                    # K^T[d, head, (t, slot)] for this block.
                    ktile = ktp.tile([128, KV, TNBLK], BF16, tag="kt")
                    kdst = ktile[:].rearrange("p h (t b e) -> p t h b e", b=2, e=8)
                    for kvc in range(KV):
                        kps = ps2.tile([128, 512], F32, tag="ps2")
                        for k in range(RC):
                            nc.tensor.matmul(
                                kps[:],
                                wk_sb[:, k, kvc * 128 : (kvc + 1) * 128],
                                rtile[:, k, :],
                                start=(k == 0),
                                stop=(k == RC - 1),
                            )
                        nc.scalar.copy(
                            kdst[:, :, :, :, kvc],
                            kps[:].rearrange("p (t a b) -> p t a b", a=8, b=2),
                        )

                    at_list = []
                    softmax_quad(blk, ktile, range(0, KV // 2), at_list)

                    # V^T[d, head, (t, slot)], same gather as K; then XBAR
                    # DMA-transpose per head to V_flat[(t,slot), d].
                    vht = vtp.tile([128, KV, TNBLK], BF16, tag="vht")
                    vdst = vht[:].rearrange("p h (t b e) -> p t h b e", b=2, e=8)
                    for kvc in range(KV):
                        vps = ps2.tile([128, 512], F32, tag="ps2")
                        for k in range(RC):
                            nc.tensor.matmul(
                                vps[:],
                                wv_sb[:, k, kvc * 128 : (kvc + 1) * 128],
                                rtile[:, k, :],
                                start=(k == 0),
                                stop=(k == RC - 1),
                            )
                        nc.scalar.copy(
                            vdst[:, :, :, :, kvc],
                            vps[:].rearrange("p (t a b) -> p t a b", a=8, b=2),
                        )
                    vflat = vtp.tile([128, KV, TNBLK // 128, D], BF16, tag="vflat")
                    for kv in range(KV):
                        nc.sync.dma_start(
                            vflat[:, kv, :, :], vht[:, kv, :], transpose=True
                        )

                    softmax_quad(blk, ktile, range(KV // 2, KV), at_list)

                    if prev is not None:
                        attn_weighted_v(*prev)
                    prev = (blk, vflat, at_list)

                attn_weighted_v(*prev)

            # ---- Stage 3: out[t, hid] = attnout^T-chunks.T x WoT ----
            with (
                tc.tile_pool(name="wo", bufs=2) as wop,
                tc.tile_pool(name="osb", bufs=3) as osp,
                tc.tile_pool(name="ps3", bufs=2, space="PSUM") as ps3,
            ):
                for f in range(HID // 512):
                    wsl = wop.tile([128, KC, 512], BF16, tag="wo")
                    if f == 0:
                        # Sub-chunked so the first O-proj matmuls start
                        # after ~1.6us instead of a full-slab 12.6us wait.
                        for kc in range(8):
                            nc.sync.dma_start(
                                wsl[:, 4 * kc : 4 * kc + 4, :],
                                wo_r[:, 4 * kc : 4 * kc + 4, 0:512],
                            )
                    else:
                        nc.sync.dma_start(wsl[:], wo_r[:, :, f * 512 : (f + 1) * 512])
                    for m in range(T // 128):
                        ops_ = ps3.tile([128, 512], F32, tag="ps3")
                        for k in range(KC):
                            nc.tensor.matmul(
                                ops_[:],
                                aoT_sb[:, k, m * 128 : (m + 1) * 128],
                                wsl[:, k, :],
                                start=(k == 0),
                                stop=(k == KC - 1),
                            )
                        ob = osp.tile([128, 512], F32, tag="ob")
                        nc.scalar.copy(ob[:], ops_[:])
                        nc.sync.dma_start(out_r[:, m, f * 512 : (f + 1) * 512], ob[:])

    nc.compile()
    return nc


def _get_nc():
    global _NC
    if _NC is None:
        _NC = _build_program()
    return _NC


def _mask_neg() -> np.ndarray:
    rows = np.arange(128)[:, None]
    cols = np.arange(512)[None, :]
    return np.where(cols // N == rows % 32, 0.0, MASK_NEG).astype(np.float32)


def build_in_maps(hidden_states, retrieved_hidden_states, Wq, Wk, Wv, Wo):
    """Host-side sharding: pre-transpose + bf16-cast, slice tokens per core."""
    bf = ml_dtypes.bfloat16
    h2 = np.asarray(hidden_states, dtype=np.float32).reshape(TOK, HID).astype(bf)
    r2 = (
        np.asarray(retrieved_hidden_states, dtype=np.float32)
        .reshape(TOK * N, RH)
        .astype(bf)
    )
    wqT = np.ascontiguousarray(np.asarray(Wq, dtype=np.float32).astype(bf).T)
    wkT = np.ascontiguousarray(np.asarray(Wk, dtype=np.float32).astype(bf).T)
    wvT = np.ascontiguousarray(np.asarray(Wv, dtype=np.float32).astype(bf).T)
    woT = np.ascontiguousarray(np.asarray(Wo, dtype=np.float32).astype(bf).T)
    mneg = _mask_neg()

    in_maps = []
    for i in range(NCORES):
        hT_i = np.ascontiguousarray(h2[i * T : (i + 1) * T].T)
        rT_i = np.ascontiguousarray(r2[i * TN : (i + 1) * TN].T)
        in_maps.append(
            {
                "hT": hT_i,
                "rT": rT_i,
                "wqT": wqT,
                "wkT": wkT,
                "wvT": wvT,
                "woT": woT,
                "mneg": mneg,
            }
        )
    return in_maps


def kernel(hidden_states, retrieved_hidden_states, Wq, Wk, Wv, Wo):
    nc = _get_nc()
    in_maps = build_in_maps(
        hidden_states, retrieved_hidden_states, Wq, Wk, Wv, Wo
    )
    res = run_bass_kernel_spmd(nc, in_maps, core_ids=list(range(NCORES)))
    outs = [res.results[i]["out"] for i in range(NCORES)]
    full = np.concatenate(outs, axis=0).reshape(B, S, HID)
    return full

